# revision 5
# baseline (speedup 1.0000x reference)
"""Trainium2 Bass kernel for nn_DeconvLayer (cascaded order-16 IIR along rows).

Math: reference applies a causal order-16 linear recurrence with taps h
(then again with reversed taps) along each row of a [4096, 4096] f32 matrix,
with the first K=16 outputs forced to zero and x[i] entering only for i >= K.

Equivalent to  y = g (*) x_masked  where x_masked zeroes columns 0..15 and
g is the rapidly decaying impulse response of the cascaded filter.  Each
128-wide output tile takes taps [0, m] from its own input tile (triangular
Toeplitz G_loc) plus taps [m+1, ..] from the previous tile via a deep slab
G_deep of width S=32; minimum tap coverage is m+1 >= 33, adding ~1.7e-3 L2
(the e3m4 output rounding of 1.34e-2 dominates; tolerance is 2e-2).

Per-core layout (rows sharded 512/core across 8 cores):
  - dma_start_transpose loads x fp16 HBM -> SBUF already time-major
    (U[t, b, r] = x[r, 128 b + t]); no PE transposes, no PSUM staging for
    the input at all.  Units split across SP and ACT HWDGE queues.
  - PE runs only the Toeplitz conv matmuls (f32 PSUM accum), ~9 us.
  - DVE + ACT cast-copy PSUM f32 -> SBUF fp8e3 (e3m4).
  - Pool (SWDGE) stores the e3m4 output; host upcasts to f32.
"""

import os
import time

import numpy as np

# the trace path needs antenv.axon_hooks, absent in this container; make
# sure a stray BASS_TRACE in the caller's env can't break execution
os.environ.setdefault("BASS_NEVER_TRACE", "1")

import concourse.bass as bass
import concourse.mybir as mybir
from concourse.bass_utils import run_bass_kernel_spmd
from concourse.tile import TileContext

N_CORES = 8
ROWS = 4096
COLS = 4096
ROWS_PER_CORE = ROWS // N_CORES  # 512
K_TAPS = 16
T_FIR = 256   # taps used when building g (covers everything above f32 noise)
S = 32        # deep-slab width: taps [m+1, 127+S] reach back into tile b-1
NT = COLS // 128  # 32 time tiles per row

_F16 = mybir.dt.float16
_F32 = mybir.dt.float32
_F8 = mybir.dt.float8e3

# schedule knobs
P_PO = 3      # [128, 1024] f32 PSUM tiles in flight (2 banks each)
P_U = 2       # [128, 32, 128] fp16 transposed-input tiles in flight
P_Y = 3       # [128, 2048] fp8 output tiles in flight
# dma_transpose units are [128, 1024] quarters of a row chunk, 16 total;
# ACT takes these unit indices (chunk*4 + quarter), SP the rest
ACT_UNITS = {0, 1, 4, 8}
# PSUM->SBUF cast-copy engine per copy index (16 copies): 1 = ACT, 0 = DVE
ACT_COPIES = {1, 3, 5, 8, 10, 12, 14}


def _impulse_response(h: np.ndarray, n: int) -> np.ndarray:
    """Impulse response of v[i] = x[i] + sum_j h[j] v[i-1-j], float64."""
    g = np.zeros(n, np.float64)
    g[0] = 1.0
    K = len(h)
    for t in range(1, n):
        lo = max(0, t - K)
        g[t] = np.dot(h[: t - lo], g[t - 1 : lo - 1 if lo > 0 else None : -1])
    return g


def _build_g_cat(h32: np.ndarray) -> np.ndarray:
    """[128, 128 + S] fp16 Toeplitz slabs [G_loc | G_deep].

    G_loc[k, m]  = g[m - k]        (own-tile taps [0, m], all 128 cols)
    G_deep[k, m] = g[128 + m - k]  (prev-tile taps [m+1, 127+S], first S cols)
    """
    h = h32.astype(np.float64)
    g1 = _impulse_response(h, T_FIR)
    g2 = _impulse_response(h[::-1], T_FIR)
    gc = np.convolve(g1, g2)[:T_FIR]
    kk = np.arange(128)[:, None]
    mm = np.arange(128)[None, :]
    g_loc = np.where(mm - kk >= 0, gc[np.clip(mm - kk, 0, T_FIR - 1)], 0.0)
    mm2 = np.arange(S)[None, :]
    t2 = 128 + mm2 - kk
    g_deep = np.where(
        (t2 >= 0) & (t2 < 128 + S), gc[np.clip(t2, 0, T_FIR - 1)], 0.0
    )
    return np.concatenate([g_loc, g_deep], axis=1).astype(np.float16)


def _build_program(legalize: bool = True) -> bass.Bass:
    """Per-core program: transpose-load -> conv matmuls -> fp8 cast -> store."""
    nc = bass.Bass()
    x = nc.dram_tensor("x", [ROWS_PER_CORE, COLS], _F16, kind="ExternalInput")
    g = nc.dram_tensor("g", [128, 128 + S], _F16, kind="ExternalInput")
    y = nc.dram_tensor("y", [ROWS_PER_CORE, COLS], _F8, kind="ExternalOutput")

    with TileContext(nc) as tc:
        with (
            tc.tile_pool(name="cpool", bufs=1) as cpool,
            tc.tile_pool(name="upool", bufs=P_U) as upool,
            tc.tile_pool(name="popool", bufs=P_PO, space="PSUM") as popool,
            tc.tile_pool(name="ypool", bufs=P_Y) as ypool,
        ):
            gt = cpool.tile([128, 128 + S], _F16, tag="g")
            nc.sync.dma_start(gt[:], g[:])

            n_copy = 0
            for rc in range(4):
                rs = slice(128 * rc, 128 * (rc + 1))
                ut = upool.tile([128, NT, 128], _F16, tag="u")
                for uu in range(4):
                    unit = 4 * rc + uu
                    eng = nc.scalar if unit in ACT_UNITS else nc.sync
                    eng.dma_start_transpose(
                        ut[:, 8 * uu : 8 * (uu + 1), :],
                        x[rs, 1024 * uu : 1024 * (uu + 1)],
                    )
                for pg in range(2):  # output panels of 2048 cols
                    yp = ypool.tile([128, 2048], _F8, tag="y")
                    for half in range(2):  # 1024-col psum tiles
                        q2 = 2 * pg + half
                        pt = popool.tile([128, 1024], _F32, tag="po")
                        # each 512-f32 bank of the 1024-wide tile is its own
                        # complete start/stop accumulation group (the PSUM
                        # zero region is one 2 KB bank per partition); own
                        # matmuls first, then the deep slabs accumulate
                        for bank in range(2):
                            plan = []
                            for j in range(4 * bank, 4 * bank + 4):
                                plan.append((128 * j, 128, 8 * q2 + j, 0))
                            for j in range(4 * bank, 4 * bank + 4):
                                if 8 * q2 + j - 1 >= 0:
                                    plan.append(
                                        (128 * j, S, 8 * q2 + j - 1, 128)
                                    )
                            for i, (col, w, b, goff) in enumerate(plan):
                                nc.tensor.matmul(
                                    pt[:, col : col + w],
                                    lhsT=ut[:, b, :],
                                    rhs=gt[:, goff : goff + w],
                                    start=(i == 0),
                                    stop=(i == len(plan) - 1),
                                )
                        dst = yp[:, 1024 * half : 1024 * (half + 1)]
                        if n_copy in ACT_COPIES:
                            nc.scalar.copy(dst, pt[:])
                        else:
                            nc.vector.tensor_copy(dst, pt[:])
                        n_copy += 1
                    c0 = 2048 * pg
                    nc.gpsimd.dma_start(y[rs, c0 : c0 + 2048], yp[:])
    if legalize:
        _legalize_waits(nc)
    return nc


def _legalize_waits(nc: bass.Bass) -> None:
    """This toolchain's walrus accepts at most ONE semaphore wait per
    instruction (Drain/EventSemaphore excepted), but Tile's semaphore
    assignment freely emits 2-3. Hoist extra waits onto injected same-engine
    NoOps placed immediately before the instruction — engines execute their
    stream serially (and a DMA trigger precedes its descriptor execution),
    so waiting earlier on the same engine preserves semantics.
    """
    for fn in nc.m.functions:
        for blk in fn.blocks:
            out = []
            changed = False
            for i in blk.instructions:
                tn = type(i).__name__
                si = i.sync_info
                cap = 2 if tn == "InstEventSemaphore" else 1
                if si is not None and len(si.on_wait) > cap:
                    waits = list(si.on_wait)
                    for w in waits[:-cap]:
                        out.append(
                            mybir.InstNoOp(
                                name=nc.get_next_instruction_name(),
                                ins=[],
                                outs=[],
                                engine=i.engine,
                                sync_info=mybir.SyncInfo(
                                    on_wait=[w], on_update=[]
                                ),
                            )
                        )
                    i.sync_info = mybir.SyncInfo(
                        on_wait=waits[-cap:], on_update=list(si.on_update)
                    )
                    changed = True
                out.append(i)
            if changed:
                blk.instructions = out


_PROGRAM = None


def kernel(**inputs: np.ndarray) -> np.ndarray:
    global _PROGRAM
    x = np.asarray(inputs["inputs"], dtype=np.float32)
    h = np.asarray(inputs["kernel"], dtype=np.float32)[0]
    assert x.shape == (ROWS, COLS) and h.shape == (K_TAPS,)

    g_cat = _build_g_cat(h)
    xm = x.astype(np.float16)
    xm[:, :K_TAPS] = 0

    if _PROGRAM is None:
        _PROGRAM = _build_program()

    in_maps = [
        {
            "x": xm[ROWS_PER_CORE * c : ROWS_PER_CORE * (c + 1)],
            "g": g_cat,
        }
        for c in range(N_CORES)
    ]
    # the axon-proxied device occasionally reports a transient
    # NRT_EXEC_UNIT_UNRECOVERABLE; a retry succeeds
    last_err = None
    for _ in range(3):
        try:
            res = run_bass_kernel_spmd(
                _PROGRAM, in_maps, list(range(N_CORES))
            ).results
            break
        except Exception as e:  # noqa: BLE001
            last_err = e
            time.sleep(2.0)
    else:
        raise last_err
    out = np.concatenate([res[c]["y"] for c in range(N_CORES)], axis=0)
    return out.astype(np.float32)


# revision 7
# speedup vs baseline: 1.1894x; 1.1894x over previous
"""Trainium2 Bass kernel for nn_DeconvLayer (cascaded order-16 IIR along rows).

Math: reference applies a causal order-16 linear recurrence with taps h
(then again with reversed taps) along each row of a [4096, 4096] f32 matrix,
with the first K=16 outputs forced to zero and x[i] entering only for i >= K.

Equivalent to  y = g (*) x_masked  where x_masked zeroes columns 0..15 and
g is the rapidly decaying impulse response of the cascaded filter.  Each
128-wide output tile takes taps [0, m] from its own input tile (triangular
Toeplitz G_loc) plus taps [m+1, ..] from the previous tile via a deep slab
G_deep of width S=32; minimum tap coverage is m+1 >= 33, adding ~1.7e-3 L2
(the e3m4 output rounding of 1.34e-2 dominates; tolerance is 2e-2).

Per-core layout (rows sharded 512/core across 8 cores):
  - dma_start_transpose loads x fp16 HBM -> SBUF already time-major
    (U[t, b, r] = x[r, 128 b + t]); no PE transposes, no PSUM staging for
    the input at all.  Units split across SP and ACT HWDGE queues.
  - PE runs only the Toeplitz conv matmuls (f32 PSUM accum), ~9 us.
  - DVE + ACT cast-copy PSUM f32 -> SBUF fp8e3 (e3m4).
  - Pool (SWDGE) stores the e3m4 output; host upcasts to f32.
"""

import os
import time

import numpy as np

# the trace path needs antenv.axon_hooks, absent in this container; make
# sure a stray BASS_TRACE in the caller's env can't break execution
os.environ.setdefault("BASS_NEVER_TRACE", "1")

import concourse.bass as bass
import concourse.mybir as mybir
from concourse.bass_utils import run_bass_kernel_spmd
from concourse.tile import TileContext

N_CORES = 8
ROWS = 4096
COLS = 4096
ROWS_PER_CORE = ROWS // N_CORES  # 512
K_TAPS = 16
T_FIR = 256   # taps used when building g (covers everything above f32 noise)
S = 32        # deep-slab width: taps [m+1, 127+S] reach back into tile b-1
NT = COLS // 128  # 32 time tiles per row

_F16 = mybir.dt.float16
_F32 = mybir.dt.float32
_F8 = mybir.dt.float8e3

# schedule knobs
P_PO = 4      # [128, 1024] f32 PSUM tiles in flight (2 banks each)
P_U = 3       # [128, 32, 128] fp16 transposed-input tiles in flight
P_Y = 3       # [128, 2048] fp8 output tiles in flight
# dma_transpose units are [128, 2048] halves of a row chunk, 8 total --
# kept to <= 8 so each HWDGE DMA gets its own queue and consumers wait on
# exact per-queue counters instead of conservative whole-queue prefixes.
# ACT takes these unit indices (chunk*2 + half), SP the rest.
ACT_UNITS = {1, 3}
# PSUM->SBUF cast-copy engine per copy index (16 copies): ACT vs DVE
ACT_COPIES = {1, 3, 5, 8, 10, 12, 14}


def _impulse_response(h: np.ndarray, n: int) -> np.ndarray:
    """Impulse response of v[i] = x[i] + sum_j h[j] v[i-1-j], float64."""
    g = np.zeros(n, np.float64)
    g[0] = 1.0
    K = len(h)
    for t in range(1, n):
        lo = max(0, t - K)
        g[t] = np.dot(h[: t - lo], g[t - 1 : lo - 1 if lo > 0 else None : -1])
    return g


def _build_g_cat(h32: np.ndarray) -> np.ndarray:
    """[128, 128 + S] fp16 Toeplitz slabs [G_loc | G_deep].

    G_loc[k, m]  = g[m - k]        (own-tile taps [0, m], all 128 cols)
    G_deep[k, m] = g[128 + m - k]  (prev-tile taps [m+1, 127+S], first S cols)
    """
    h = h32.astype(np.float64)
    g1 = _impulse_response(h, T_FIR)
    g2 = _impulse_response(h[::-1], T_FIR)
    gc = np.convolve(g1, g2)[:T_FIR]
    kk = np.arange(128)[:, None]
    mm = np.arange(128)[None, :]
    g_loc = np.where(mm - kk >= 0, gc[np.clip(mm - kk, 0, T_FIR - 1)], 0.0)
    mm2 = np.arange(S)[None, :]
    t2 = 128 + mm2 - kk
    g_deep = np.where(
        (t2 >= 0) & (t2 < 128 + S), gc[np.clip(t2, 0, T_FIR - 1)], 0.0
    )
    return np.concatenate([g_loc, g_deep], axis=1).astype(np.float16)


def _build_program(legalize: bool = True) -> bass.Bass:
    """Per-core program: transpose-load -> conv matmuls -> fp8 cast -> store."""
    nc = bass.Bass()
    x = nc.dram_tensor("x", [ROWS_PER_CORE, COLS], _F16, kind="ExternalInput")
    g = nc.dram_tensor("g", [128, 128 + S], _F16, kind="ExternalInput")
    y = nc.dram_tensor("y", [ROWS_PER_CORE, COLS], _F8, kind="ExternalOutput")

    with TileContext(nc) as tc:
        with (
            tc.tile_pool(name="cpool", bufs=1) as cpool,
            tc.tile_pool(name="upool", bufs=P_U) as upool,
            tc.tile_pool(name="popool", bufs=P_PO, space="PSUM") as popool,
            tc.tile_pool(name="ypool", bufs=P_Y) as ypool,
        ):
            gt = cpool.tile([128, 128 + S], _F16, tag="g")
            nc.sync.dma_start(gt[:], g[:])

            n_copy = 0
            for rc in range(4):
                rs = slice(128 * rc, 128 * (rc + 1))
                ut = upool.tile([128, NT, 128], _F16, tag="u")
                for uu in range(2):
                    unit = 2 * rc + uu
                    eng = nc.scalar if unit in ACT_UNITS else nc.sync
                    eng.dma_start_transpose(
                        ut[:, 16 * uu : 16 * (uu + 1), :],
                        x[rs, 2048 * uu : 2048 * (uu + 1)],
                    )
                for pg in range(2):  # output panels of 2048 cols
                    yp = ypool.tile([128, 2048], _F8, tag="y")
                    for half in range(2):  # 1024-col psum tiles
                        q2 = 2 * pg + half
                        pt = popool.tile([128, 1024], _F32, tag="po")
                        # each 512-f32 bank of the 1024-wide tile is its own
                        # complete start/stop accumulation group (the PSUM
                        # zero region is one 2 KB bank per partition); own
                        # matmuls first, then the deep slabs accumulate
                        for bank in range(2):
                            plan = []
                            for j in range(4 * bank, 4 * bank + 4):
                                plan.append((128 * j, 128, 8 * q2 + j, 0))
                            for j in range(4 * bank, 4 * bank + 4):
                                if 8 * q2 + j - 1 >= 0:
                                    plan.append(
                                        (128 * j, S, 8 * q2 + j - 1, 128)
                                    )
                            for i, (col, w, b, goff) in enumerate(plan):
                                nc.tensor.matmul(
                                    pt[:, col : col + w],
                                    lhsT=ut[:, b, :],
                                    rhs=gt[:, goff : goff + w],
                                    start=(i == 0),
                                    stop=(i == len(plan) - 1),
                                )
                        dst = yp[:, 1024 * half : 1024 * (half + 1)]
                        if n_copy in ACT_COPIES:
                            nc.scalar.copy(dst, pt[:])
                        else:
                            nc.vector.tensor_copy(dst, pt[:])
                        n_copy += 1
                    c0 = 2048 * pg
                    nc.gpsimd.dma_start(y[rs, c0 : c0 + 2048], yp[:])
    if legalize:
        _legalize_waits(nc)
    return nc


def _legalize_waits(nc: bass.Bass) -> None:
    """This toolchain's walrus accepts at most ONE semaphore wait per
    instruction (Drain/EventSemaphore excepted), but Tile's semaphore
    assignment freely emits 2-3. Hoist extra waits onto injected same-engine
    NoOps placed immediately before the instruction — engines execute their
    stream serially (and a DMA trigger precedes its descriptor execution),
    so waiting earlier on the same engine preserves semantics.
    """
    for fn in nc.m.functions:
        for blk in fn.blocks:
            out = []
            changed = False
            for i in blk.instructions:
                tn = type(i).__name__
                si = i.sync_info
                cap = 2 if tn == "InstEventSemaphore" else 1
                if si is not None and len(si.on_wait) > cap:
                    waits = list(si.on_wait)
                    for w in waits[:-cap]:
                        out.append(
                            mybir.InstNoOp(
                                name=nc.get_next_instruction_name(),
                                ins=[],
                                outs=[],
                                engine=i.engine,
                                sync_info=mybir.SyncInfo(
                                    on_wait=[w], on_update=[]
                                ),
                            )
                        )
                    i.sync_info = mybir.SyncInfo(
                        on_wait=waits[-cap:], on_update=list(si.on_update)
                    )
                    changed = True
                out.append(i)
            if changed:
                blk.instructions = out


_PROGRAM = None


def kernel(**inputs: np.ndarray) -> np.ndarray:
    global _PROGRAM
    x = np.asarray(inputs["inputs"], dtype=np.float32)
    h = np.asarray(inputs["kernel"], dtype=np.float32)[0]
    assert x.shape == (ROWS, COLS) and h.shape == (K_TAPS,)

    g_cat = _build_g_cat(h)
    xm = x.astype(np.float16)
    xm[:, :K_TAPS] = 0

    if _PROGRAM is None:
        _PROGRAM = _build_program()

    in_maps = [
        {
            "x": xm[ROWS_PER_CORE * c : ROWS_PER_CORE * (c + 1)],
            "g": g_cat,
        }
        for c in range(N_CORES)
    ]
    # the axon-proxied device occasionally reports a transient
    # NRT_EXEC_UNIT_UNRECOVERABLE; a retry succeeds
    last_err = None
    for _ in range(3):
        try:
            res = run_bass_kernel_spmd(
                _PROGRAM, in_maps, list(range(N_CORES))
            ).results
            break
        except Exception as e:  # noqa: BLE001
            last_err = e
            time.sleep(2.0)
    else:
        raise last_err
    out = np.concatenate([res[c]["y"] for c in range(N_CORES)], axis=0)
    return out.astype(np.float32)
